# revision 1
# baseline (speedup 1.0000x reference)
"""Trainium2 Bass kernel for nn_BaseDecoder (6-layer transformer decoder).

Contract: kernel(**inputs) takes FULL numpy inputs (as in reference.setup_inputs())
and returns the FULL output [B, LT, V] float32.

Sharding: pure data-parallel over batch -- 2 batches per core x 8 cores, no
collectives. All weights replicated per core (fp16, pre-transposed on host).

Per-core device program (per layer):
  self-attn:  qkT = Wqk-tiles.T @ xT (q pre-scaled by 1/sqrt(D));  v token-major
              s^T[k,q] = K^T-slice.T @ Q^T;  expT = exp(s^T) * exp(bias)^T
              u_aug^T[d+1,q] = [v|1]-slice.T @ expT  (row d = softmax denom)
              o^T = u^T * bcast(1/denom);  x += Wo-proj(o^T);  x = LN(x)
  cross-attn: same with kv from memory, no distance bias
  ffn:        h^T = gelu(W1.T-tiles @ xT);  x += W2-proj(h^T);  x = LN(x)
Matmul operands fp16 (1 cyc/row on PE), accumulation fp32 in PSUM.
LayerNorm on the fp32 token-major residual via bn_stats/bn_aggr; the LN output
replaces the residual (post-LN architecture) and is also transposed (PE
transpose) into the feature-major fp16 activation buffer feeding matmuls.
"""

import os
import sys

import numpy as np

for _p in ("/root/.axon_site/_ro/trn_rl_repo", "/opt/trn_rl_repo"):
    if os.path.isdir(_p) and _p not in sys.path:
        sys.path.append(_p)

import bass_rust
import concourse.bass as bass
import concourse.mybir as mybir
import concourse.tile as tile
from concourse.bass_utils import run_bass_kernel_spmd
from concourse.masks import make_identity

# model dims (hardcoded from the problem spec)
B, LT, LM, E, H, F, NL, V = 16, 512, 512, 1024, 16, 4096, 6, 70
D = E // H                      # 64 head dim
NCORES = 8
BL = B // NCORES                # 2 batches per core
T = BL * LT                     # 1024 tokens per core
EC = E // 128                   # 8 feature chunks
TC = T // 128                   # 8 token chunks
FC = F // 128                   # 32 ffn chunks
KC = LT // 128                  # 4 key chunks per batch

F16 = mybir.dt.float16
F32 = mybir.dt.float32

_N_LAYERS = int(os.environ.get("CC_NL", str(NL)))


class _TC(tile.TileContext):
    """TileContext whose tail drain splits its sem waits across NOPs.

    The pinned walrus build encodes at most one sync-wait per CTRL
    instruction; the stock tail drain carries one wait per DMA queue and
    fails codegen. Equivalent semantics: the drain keeps the first wait and
    each extra wait rides its own SP NOP before the all-engine barrier.
    """

    def _drain_and_barrier(self, tick_clock, wait_clock):
        from concourse.tile import ScopedClock

        drain_inst = self.nc.sync.drain()
        wait_clock.add_sem_waits(
            drain_inst.ins, ScopedClock({None: tick_clock.global_clock})
        )
        si = drain_inst.ins.sync_info
        if si is not None and len(si.on_wait) > 1:
            waits = list(si.on_wait)
            si.on_wait = waits[:1]
            drain_inst.ins.sync_info = si
            for w in waits[1:]:
                nop = self.nc.sync.nop(nofuse=True, hint="drain_split")
                nop.ins.sync_info = bass_rust.SyncInfo(on_wait=[w], on_update=[])
        self.nc.all_engine_barrier()
        popped = self.nc._tile_sem_poison_stack.pop()
        assert popped is self._sem_poison
        self.nc.clear_and_free_semaphores(list(self.sems.allocated().values()))
        self.nc.all_engine_barrier()


def _legalize_multi_waits(nc):
    """Split multi-wait instructions for the 1-wait-per-instruction walrus.

    Extra sem-waits move onto same-engine NOPs inserted immediately before
    the instruction; engine queues are FIFO (and SP issues HWDGE descriptors
    in order), so semantics are preserved.
    """
    n_split = 0
    for fn in nc.m.functions:
        for blk in fn.blocks:
            out = []
            changed = False
            for ins in blk.instructions:
                si = getattr(ins, "sync_info", None)
                if si is not None and len(si.on_wait) > 1:
                    waits = list(si.on_wait)
                    for j, w in enumerate(waits[:-1]):
                        nop = mybir.InstNoOp(name=f"{ins.name}-ws{j}", ins=[], outs=[])
                        nop.engine = ins.engine
                        nop.sync_info = bass_rust.SyncInfo(on_wait=[w], on_update=[])
                        out.append(nop)
                        n_split += 1
                    si.on_wait = [waits[-1]]
                    ins.sync_info = si
                    changed = True
                out.append(ins)
            if changed:
                blk.instructions = out
    return n_split


def _build_program(flags):
    """Build the per-core Bass program. flags: dict of host-derived config."""
    import contextlib

    nc = bass.Bass("TRN2")
    nl = flags["n_layers"]

    # ---- DRAM I/O ----
    x0 = nc.dram_tensor("x0", [T, E], F32, kind="ExternalInput")
    x0T = nc.dram_tensor("x0T", [E, T], F16, kind="ExternalInput")
    memT_d = nc.dram_tensor("memT", [E, T], F16, kind="ExternalInput")
    eb = nc.dram_tensor("eb", [BL, H, LT, LT], F16, kind="ExternalInput")
    wqk = nc.dram_tensor("wqk", [nl, E, 2 * E], F16, kind="ExternalInput")
    wv_d = nc.dram_tensor("wv", [nl, E, E], F16, kind="ExternalInput")
    wo_d = nc.dram_tensor("wo", [nl, E, E], F16, kind="ExternalInput")
    cqk = nc.dram_tensor("cqk", [nl, E, 2 * E], F16, kind="ExternalInput")
    cv_d = nc.dram_tensor("cv", [nl, E, E], F16, kind="ExternalInput")
    co_d = nc.dram_tensor("co", [nl, E, E], F16, kind="ExternalInput")
    w1_d = nc.dram_tensor("w1", [nl, E, F], F16, kind="ExternalInput")
    w2_d = nc.dram_tensor("w2", [nl, F, E], F16, kind="ExternalInput")
    wgen = nc.dram_tensor("wgen", [E, V], F16, kind="ExternalInput")
    eb0_d = nc.dram_tensor("eb0", [BL, H, LT, LT], F16, kind="ExternalInput")
    out_d = nc.dram_tensor("out", [T, V], F32, kind="ExternalOutput")

    DBG = flags.get("dbg_stage", 99)
    dbg_t = {}
    if DBG < 99:
        dbg_t["xres"] = nc.dram_tensor("dbg_xres", [T, E], F32, kind="ExternalOutput")
        dbg_t["a"] = nc.dram_tensor("dbg_a", [E, T], F16, kind="ExternalOutput")
        dbg_t["b"] = nc.dram_tensor("dbg_b", [E, T], F16, kind="ExternalOutput")
        dbg_t["v"] = nc.dram_tensor("dbg_v", [T, H * 65], F16, kind="ExternalOutput")
        dbg_t["e"] = nc.dram_tensor("dbg_e", [LT, LT], F16, kind="ExternalOutput")
        dbg_t["u"] = nc.dram_tensor("dbg_u", [128, 512], F32, kind="ExternalOutput")

    ebc_d = None
    if flags["cross_mask"]:
        ebc_d = nc.dram_tensor("ebc", [BL, LM], F32, kind="ExternalInput")
    pb_d = {}
    for nm, shape in [("b_o", [nl, E]), ("cb_o", [nl, E]), ("b_1", [nl, F]),
                      ("b_2", [nl, E]), ("b_gen", [V])]:
        if flags[nm]:
            pb_d[nm] = nc.dram_tensor(nm, shape, F32, kind="ExternalInput")
    ln_d = {}
    for nm in ("ln1", "ln2", "ln3", "lnf"):
        if flags[nm]:
            shape = [E] if nm == "lnf" else [nl, E]
            ln_d[nm + "_g"] = nc.dram_tensor(nm + "_g", shape, F32, kind="ExternalInput")
            ln_d[nm + "_b"] = nc.dram_tensor(nm + "_b", shape, F32, kind="ExternalInput")

    with _TC(nc) as tc, contextlib.ExitStack() as ctx:
        root = ctx.enter_context(tc.tile_pool(name="root", bufs=1))
        consts = ctx.enter_context(tc.tile_pool(name="consts", bufs=1))
        stats_p = ctx.enter_context(tc.tile_pool(name="stats", bufs=4))
        xn_p = ctx.enter_context(tc.tile_pool(name="xn", bufs=3))
        ps_tp = ctx.enter_context(tc.tile_pool(name="ps_tp", bufs=2, space="PSUM"))
        ps_proj = ctx.enter_context(tc.tile_pool(name="ps_proj", bufs=2, space="PSUM"))

        # persistent state
        xres = [root.tile([128, E], F32, tag=f"xres{c}", name=f"xres{c}") for c in range(TC)]
        xT = [root.tile([128, T], F16, tag=f"xT{r}", name=f"xT{r}") for r in range(EC)]
        memT = [root.tile([128, T], F16, tag=f"memT{r}", name=f"memT{r}") for r in range(EC)]

        identity = consts.tile([128, 128], F16)
        make_identity(nc, identity[:])
        eps_t = consts.tile([128, 1], F32)
        nc.vector.memset(eps_t[:], 1e-5)
        ones_row = consts.tile([1, 128], F16)
        nc.vector.memset(ones_row[:], 1.0)
        neg4 = consts.tile([128, 1], F32)
        nc.vector.memset(neg4[:], -4.0)
        ones64_f = consts.tile([1, 64], F32)
        nc.vector.memset(ones64_f[:], 1.0)
        ones64 = consts.tile([1, 64], mybir.dt.float32r)
        nc.scalar.copy(out=ones64[:], in_=ones64_f[:])

        for c in range(TC):
            nc.sync.dma_start(out=xres[c][:], in_=x0[c * 128:(c + 1) * 128, :])
        for r in range(EC):
            nc.sync.dma_start(out=xT[r][:], in_=x0T[r * 128:(r + 1) * 128, :])
            nc.sync.dma_start(out=memT[r][:], in_=memT_d[r * 128:(r + 1) * 128, :])

        ebc_sb = None
        if ebc_d is not None:
            ebc_sb = consts.tile([128, BL, KC], F32)  # [k-part, b, kc]
            for b in range(BL):
                for kc in range(KC):
                    nc.sync.dma_start(
                        out=ebc_sb[:, b, kc:kc + 1],
                        in_=ebc_d[b, kc * 128:(kc + 1) * 128][:, None])

        def bcast_row(dst, dram_ap, n):
            """DMA a [n] DRAM row broadcast to all 128 partitions."""
            bap = bass.AP(tensor=dram_ap.tensor, offset=dram_ap.offset,
                          ap=[[0, 128]] + list(dram_ap.ap))
            nc.sync.dma_start(out=dst[:, 0:n], in_=bap)

        def layernorm(lname, l, bias_bc=None):
            """xres = LN(xres [+ bias_bc]); also writes the transposed fp16
            copy of the new stream into xT (feature-major)."""
            gb = None
            if flags[lname]:
                g_bc = xn_p.tile([128, E], F32, tag="g_bc")
                b_bc = xn_p.tile([128, E], F32, tag="b_bc")
                g_ap = ln_d[lname + "_g"] if lname == "lnf" else ln_d[lname + "_g"][l]
                b_ap = ln_d[lname + "_b"] if lname == "lnf" else ln_d[lname + "_b"][l]
                bcast_row(g_bc, g_ap, E)
                bcast_row(b_bc, b_ap, E)
                gb = (g_bc, b_bc)
            for c in range(TC):
                x_c = xres[c]
                if bias_bc is not None:
                    nc.vector.tensor_add(out=x_c[:], in0=x_c[:], in1=bias_bc[:])
                st = stats_p.tile([128, 2, 6], F32, tag="bnst")
                for sg in range(2):
                    nc.vector.bn_stats(out=st[:, sg, :], in_=x_c[:, sg * 512:(sg + 1) * 512])
                mv = stats_p.tile([128, 2], F32, tag="bnmv")
                nc.vector.bn_aggr(out=mv[:], in_=st[:])
                std = stats_p.tile([128, 1], F32, tag="bnsd")
                nc.scalar.activation(out=std[:], in_=mv[:, 1:2],
                                     func=mybir.ActivationFunctionType.Sqrt,
                                     bias=eps_t[:], scale=1.0)
                nc.vector.reciprocal(out=std[:], in_=std[:])
                # residual stream becomes the LN output (post-LN architecture)
                nc.vector.tensor_scalar(
                    out=x_c[:], in0=x_c[:], scalar1=mv[:, 0:1], scalar2=std[:],
                    op0=mybir.AluOpType.subtract, op1=mybir.AluOpType.mult)
                if gb is not None:
                    nc.vector.tensor_mul(out=x_c[:], in0=x_c[:], in1=gb[0][:])
                    nc.vector.tensor_add(out=x_c[:], in0=x_c[:], in1=gb[1][:])
                xn = xn_p.tile([128, E], F16, tag="xn")
                nc.scalar.copy(out=xn[:], in_=x_c[:])
                for r in range(EC):
                    pt = ps_tp.tile([128, 128], F16, tag="tp")
                    nc.tensor.transpose(pt[:], xn[:, r * 128:(r + 1) * 128], identity[:])
                    if r % 2 == 0:
                        nc.vector.tensor_copy(out=xT[r][:, c * 128:(c + 1) * 128], in_=pt[:])
                    else:
                        nc.scalar.copy(out=xT[r][:, c * 128:(c + 1) * 128], in_=pt[:])

        def dump_ft(key, tiles):
            for r in range(EC):
                nc.sync.dma_start(out=dbg_t[key][r * 128:(r + 1) * 128, :], in_=tiles[r][:])

        def dump_xres():
            for c in range(TC):
                nc.sync.dma_start(out=dbg_t["xres"][c * 128:(c + 1) * 128, :], in_=xres[c][:])

        def attention(l, w_qk, w_v, w_o, kv_src, use_eb, pools, dbg_base=100,
                      scores_from_host=False):
            """One attention block. kv_src: xT (self) or memT (cross)."""
            (qT, kT, vpad, oT, wpool, ebp, tmp_p, exp_p, rr_p, ps_sc, ps_u,
             adram) = pools

            for c in range(TC):
                nc.vector.memset(vpad[c][:, :, 64:65], 1.0)

            # QK projection (orientation-1: output feature-major)
            if DBG < dbg_base + 1:
                return
            for half, dst in ((0, qT), (1, kT)) if not scores_from_host else ():
                src = xT if half == 0 else kv_src
                for ob in range(2):
                    wt = []
                    for r in range(EC):
                        w_sb = wpool.tile([128, 512], F16, tag="wblk")
                        nc.sync.dma_start(
                            out=w_sb[:],
                            in_=w_qk[l, r * 128:(r + 1) * 128,
                                     half * E + ob * 512: half * E + (ob + 1) * 512])
                        wt.append(w_sb)
                    for oc in range(4):
                        og = ob * 4 + oc          # o-chunk index within E
                        for tcol in range(2):
                            ps = ps_proj.tile([128, 512], F32, tag="pj")
                            for r in range(EC):
                                nc.tensor.matmul(
                                    ps[:], wt[r][:, oc * 128:(oc + 1) * 128],
                                    src[r][:, tcol * 512:(tcol + 1) * 512],
                                    start=(r == 0), stop=(r == EC - 1))
                            nc.scalar.copy(
                                out=dst[og][:, tcol * 512:(tcol + 1) * 512], in_=ps[:])

            if DBG == dbg_base + 1:
                dump_ft("a", qT); dump_ft("b", kT)
                return
            # V projection (orientation-2: output token-major into vpad)
            for ob in range(2):
                wt = []
                for r in range(EC):
                    w_sb = wpool.tile([128, 512], F16, tag="wblk")
                    nc.sync.dma_start(
                        out=w_sb[:],
                        in_=w_v[l, r * 128:(r + 1) * 128, ob * 512:(ob + 1) * 512])
                    wt.append(w_sb)
                for c in range(TC):
                    ps = ps_proj.tile([128, 512], F32, tag="pj")
                    for r in range(EC):
                        nc.tensor.matmul(
                            ps[:], kv_src[r][:, c * 128:(c + 1) * 128], wt[r][:],
                            start=(r == 0), stop=(r == EC - 1))
                    nc.scalar.copy(
                        out=vpad[c][:, ob * 8:(ob + 1) * 8, 0:64],
                        in_=ps[:].rearrange("p (h d) -> p h d", d=64))

            if DBG == dbg_base + 2:
                for c in range(TC):
                    nc.sync.dma_start(
                        out=dbg_t["v"][c * 128:(c + 1) * 128, :],
                        in_=vpad[c][:].rearrange("p h d -> p (h d)"))
                return
            # attention core, per (head-pair, batch, head). Normalization is
            # deferred: oT collects unnormalized u (v pre-scaled 1/1024 on
            # host keeps it in fp16 range), denominators collect in den_sb,
            # and one batched reciprocal + dense normalize pass follows.
            den_sb = rr_p.tile([BL * H, 512], F32, tag="den")
            for hp in range(H // 2):
              for b in range(BL):
                for hh in range(2):
                    h = 2 * hp + hh
                    r, po = hp, hh * 64
                    j = h * BL + b
                    exps = []
                    for kc in range(KC):
                        ex = exp_p.tile([128, 512], F16, tag="exp")
                        dump_this = (DBG == dbg_base + 3 and b == 0 and h == 0)
                        if scores_from_host:
                            nc.sync.dma_start(
                                out=ex[:], in_=eb0_d[b, h, kc * 128:(kc + 1) * 128, :])
                            if dump_this:
                                nc.sync.dma_start(
                                    out=dbg_t["e"][kc * 128:(kc + 1) * 128, :], in_=ex[:])
                            exps.append(ex)
                            continue
                        ps_s = ps_sc.tile([128, 512], F32, tag="sc")
                        nc.tensor.matmul(
                            ps_s[:],
                            kT[r][po:po + 64, b * 512 + kc * 128: b * 512 + (kc + 1) * 128],
                            qT[r][po:po + 64, b * 512:(b + 1) * 512],
                            start=True, stop=True)
                        if use_eb:
                            tm = tmp_p.tile([128, 512], F32, tag="tmp")
                            nc.scalar.activation(out=tm[:], in_=ps_s[:],
                                                 func=mybir.ActivationFunctionType.Exp,
                                                 bias=neg4[:])
                            ebt = ebp.tile([128, 512], F16, tag="eb")
                            nc.sync.dma_start(out=ebt[:],
                                              in_=eb[b, h, kc * 128:(kc + 1) * 128, :])
                            nc.vector.scalar_tensor_tensor(
                                out=ex[:], in0=tm[:], scalar=60000.0, in1=ebt[:],
                                op0=mybir.AluOpType.min, op1=mybir.AluOpType.mult)
                        elif ebc_sb is not None:
                            tm = tmp_p.tile([128, 512], F32, tag="tmp")
                            nc.scalar.activation(out=tm[:], in_=ps_s[:],
                                                 func=mybir.ActivationFunctionType.Exp,
                                                 bias=neg4[:])
                            nc.vector.tensor_scalar(
                                out=ex[:], in0=tm[:], scalar1=60000.0,
                                scalar2=ebc_sb[:, b, kc:kc + 1],
                                op0=mybir.AluOpType.min, op1=mybir.AluOpType.mult)
                        else:
                            nc.scalar.activation(out=ex[:], in_=ps_s[:],
                                                 func=mybir.ActivationFunctionType.Exp,
                                                 bias=neg4[:])
                        if dump_this:
                            nc.sync.dma_start(
                                out=dbg_t["e"][kc * 128:(kc + 1) * 128, :], in_=ex[:])
                        exps.append(ex)
                    pu = ps_u.tile([65, 512], F32, tag="ua")
                    for kc in range(KC):
                        nc.tensor.matmul(
                            pu[:], vpad[b * KC + kc][:, h, :], exps[kc][:],
                            start=(kc == 0), stop=(kc == KC - 1))
                    if dump_this:
                        usb = rr_p.tile([65, 512], F32, tag="usb")
                        nc.vector.tensor_copy(out=usb[:], in_=pu[:])
                        nc.sync.dma_start(out=dbg_t["u"][0:65, :], in_=usb[:])
                    den_row = rr_p.tile([1, 512], F32, tag="dr")
                    nc.vector.tensor_copy(out=den_row[:], in_=pu[64:65, :])
                    nc.sync.dma_start(out=den_sb[j:j + 1, :], in_=den_row[:])
                    nc.scalar.copy(
                        out=oT[r][po:po + 64, b * 512:(b + 1) * 512],
                        in_=pu[0:64, :])
            # batched reciprocal + dense normalize pass (DRAM round trip so the
            # per-row broadcast can use a partition-step-0 DRAM source AP)
            recips = rr_p.tile([BL * H, 512], F32, tag="rcp")
            nc.vector.reciprocal(out=recips[:], in_=den_sb[:])
            scr = adram.tile([BL * H, 512], F32, tag="scr")
            nc.sync.dma_start(out=scr[:], in_=recips[:])
            for hp in range(H // 2):
              for b in range(BL):
                for hh in range(2):
                    h = 2 * hp + hh
                    r, po = hp, hh * 64
                    j = h * BL + b
                    bc_sb = rr_p.tile([128, 512], F32, tag="bc")
                    nc.sync.dma_start(
                        out=bc_sb[po:po + 64, :],
                        in_=scr[j:j + 1, :].to_broadcast((64, 512)))
                    sl = oT[r][po:po + 64, b * 512:(b + 1) * 512]
                    nc.vector.tensor_mul(out=sl, in0=sl, in1=bc_sb[po:po + 64, :])

            if DBG == dbg_base + 3:
                dump_ft("a", oT)
                return
            # out projection (orientation-2) + residual add
            for ob in range(2):
                wt = []
                for r in range(EC):
                    w_sb = wpool.tile([128, 512], F16, tag="wblk")
                    nc.sync.dma_start(
                        out=w_sb[:],
                        in_=w_o[l, r * 128:(r + 1) * 128, ob * 512:(ob + 1) * 512])
                    wt.append(w_sb)
                for c in range(TC):
                    ps = ps_proj.tile([128, 512], F32, tag="pj")
                    for r in range(EC):
                        nc.tensor.matmul(
                            ps[:], oT[r][:, c * 128:(c + 1) * 128], wt[r][:],
                            start=(r == 0), stop=(r == EC - 1))
                    xs = xres[c][:, ob * 512:(ob + 1) * 512]
                    nc.vector.tensor_add(out=xs, in0=ps[:], in1=xs)

        def enter_attn_pools(st):
            p = st.enter_context(tc.tile_pool(name="attn_bufs", bufs=1))
            qT = [p.tile([128, T], F16, tag=f"qT{r}", name=f"qT{r}") for r in range(EC)]
            kT = [p.tile([128, T], F16, tag=f"kT{r}", name=f"kT{r}") for r in range(EC)]
            vp = [p.tile([128, H, 65], F16, tag=f"vp{c}", name=f"vp{c}") for c in range(TC)]
            oT = [p.tile([128, T], F16, tag=f"oT{r}", name=f"oT{r}") for r in range(EC)]
            wpool = st.enter_context(tc.tile_pool(name="aw", bufs=16))
            ebp = st.enter_context(tc.tile_pool(name="ebp", bufs=6))
            tmp_p = st.enter_context(tc.tile_pool(name="tmp", bufs=4))
            exp_p = st.enter_context(tc.tile_pool(name="expp", bufs=8))
            rr_p = st.enter_context(tc.tile_pool(name="rrp", bufs=3))
            adram = st.enter_context(tc.tile_pool(name="adram", bufs=2, space="DRAM"))
            ps_sc = st.enter_context(tc.tile_pool(name="ps_sc", bufs=2, space="PSUM"))
            ps_u = st.enter_context(tc.tile_pool(name="ps_u", bufs=2, space="PSUM"))
            return (qT, kT, vp, oT, wpool, ebp, tmp_p, exp_p, rr_p, ps_sc, ps_u,
                    adram)

        import contextlib as _cl
        for l in range(nl):
            with _cl.ExitStack() as st:
                pools = enter_attn_pools(st)
                attention(l, wqk, wv_d, wo_d, xT, True, pools, dbg_base=0,
                          scores_from_host=(l == 0))
                if DBG <= 4:
                    if DBG == 4:
                        dump_xres()
                    break
                bias_bc = None
                if flags["b_o"]:
                    bias_bc = xn_p.tile([128, E], F32, tag="pb_bc")
                    bcast_row(bias_bc, pb_d["b_o"][l], E)
                layernorm("ln1", l, bias_bc)
                if DBG == 5:
                    dump_xres(); dump_ft("a", xT)
                    break
                attention(l, cqk, cv_d, co_d, memT, False, pools, dbg_base=10)
                if DBG <= 14:
                    if DBG == 14:
                        dump_xres()
                    break
                bias_bc = None
                if flags["cb_o"]:
                    bias_bc = xn_p.tile([128, E], F32, tag="pb_bc")
                    bcast_row(bias_bc, pb_d["cb_o"][l], E)
                layernorm("ln2", l, bias_bc)
                if DBG == 15:
                    dump_xres(); dump_ft("a", xT)
                    break

            if DBG < 16:
                break
            with _cl.ExitStack() as st:
                hp = st.enter_context(tc.tile_pool(name="h_bufs", bufs=1))
                hT = [hp.tile([128, 512], F16, tag=f"hT{f}", name=f"hT{f}") for f in range(FC)]
                w1p = st.enter_context(tc.tile_pool(name="w1p", bufs=16))
                w2p = st.enter_context(tc.tile_pool(name="w2p", bufs=34))
                b1_sb = None
                if flags["b_1"]:
                    b1_sb = st.enter_context(tc.tile_pool(name="b1s", bufs=1)).tile([128, FC], F32, name="b1_sb")
                    for f in range(FC):
                        nc.sync.dma_start(out=b1_sb[:, f:f + 1],
                                          in_=pb_d["b_1"][l, f * 128:(f + 1) * 128][:, None])
                for half in range(2):
                    tcols = slice(half * 512, (half + 1) * 512)
                    for ob in range(8):
                        wt = []
                        for r in range(EC):
                            w_sb = w1p.tile([128, 512], F16, tag="w1b")
                            nc.sync.dma_start(
                                out=w_sb[:],
                                in_=w1_d[l, r * 128:(r + 1) * 128, ob * 512:(ob + 1) * 512])
                            wt.append(w_sb)
                        for fcc in range(4):
                            fg = ob * 4 + fcc
                            ps = ps_proj.tile([128, 512], F32, tag="pj")
                            for r in range(EC):
                                nc.tensor.matmul(
                                    ps[:], wt[r][:, fcc * 128:(fcc + 1) * 128],
                                    xT[r][:, tcols],
                                    start=(r == 0), stop=(r == EC - 1))
                            if b1_sb is not None:
                                nc.scalar.activation(
                                    out=hT[fg][:], in_=ps[:],
                                    func=mybir.ActivationFunctionType.Gelu,
                                    bias=b1_sb[:, fg:fg + 1])
                            else:
                                nc.scalar.activation(
                                    out=hT[fg][:], in_=ps[:],
                                    func=mybir.ActivationFunctionType.Gelu)
                    for oc in range(2):
                        wt2 = []
                        for f in range(FC):
                            w_sb = w2p.tile([128, 512], F16, tag="w2b")
                            nc.sync.dma_start(
                                out=w_sb[:],
                                in_=w2_d[l, f * 128:(f + 1) * 128, oc * 512:(oc + 1) * 512])
                            wt2.append(w_sb)
                        for tcl in range(4):
                            c = half * 4 + tcl
                            ps = ps_proj.tile([128, 512], F32, tag="pj")
                            for f in range(FC):
                                nc.tensor.matmul(
                                    ps[:], hT[f][:, tcl * 128:(tcl + 1) * 128], wt2[f][:],
                                    start=(f == 0), stop=(f == FC - 1))
                            xs = xres[c][:, oc * 512:(oc + 1) * 512]
                            nc.vector.tensor_add(out=xs, in0=ps[:], in1=xs)
                bias_bc = None
                if flags["b_2"]:
                    bias_bc = xn_p.tile([128, E], F32, tag="pb_bc")
                    bcast_row(bias_bc, pb_d["b_2"][l], E)
                layernorm("ln3", l, bias_bc)
            if DBG < 99:
                dump_xres()
                break

        # final LN + generator head
        if DBG >= 99:
            layernorm("lnf", 0, None)
        with contextlib.ExitStack() as st:
          if DBG >= 99:
            gp = st.enter_context(tc.tile_pool(name="gp", bufs=1))
            op = st.enter_context(tc.tile_pool(name="op", bufs=3))
            wt = []
            for r in range(EC):
                w_sb = gp.tile([128, V], F16, tag=f"wg{r}", name=f"wg{r}")
                nc.sync.dma_start(out=w_sb[:], in_=wgen[r * 128:(r + 1) * 128, :])
                wt.append(w_sb)
            bgen_bc = None
            if flags["b_gen"]:
                bgen_bc = gp.tile([128, V], F32, tag="bgen")
                bcast_row(bgen_bc, pb_d["b_gen"], V)
            for c in range(TC):
                ps = ps_proj.tile([128, 512], F32, tag="pj", name="pjg")[:, 0:V]
                for r in range(EC):
                    nc.tensor.matmul(
                        ps[:], xT[r][:, c * 128:(c + 1) * 128], wt[r][:],
                        start=(r == 0), stop=(r == EC - 1))
                osb = op.tile([128, V], F32, tag="osb")
                if bgen_bc is not None:
                    nc.vector.tensor_add(out=osb[:], in0=ps[:], in1=bgen_bc[:])
                else:
                    nc.vector.tensor_copy(out=osb[:], in_=ps[:])
                nc.sync.dma_start(out=out_d[c * 128:(c + 1) * 128, :], in_=osb[:])

    _legalize_multi_waits(nc)
    return nc


# ---------------------------------------------------------------------------
# host side
# ---------------------------------------------------------------------------

_CACHE = {}


def _prep_host(inputs):
    """Host-side preprocessing. Returns (flags, shared_arrays, percore_arrays)."""
    nl = _N_LAYERS
    f32 = np.float32
    f16 = np.float16

    seq = np.asarray(inputs["sequences"])
    dsq = np.asarray(inputs["distance_squares"])
    memory = np.asarray(inputs["memory"], dtype=f32)
    kpm = np.asarray(inputs["memory_key_padding_mask"])
    tok_emb = np.asarray(inputs["tok_emb"], dtype=f32)
    dist_emb = np.asarray(inputs["dist_emb"], dtype=f32)

    flags = {"n_layers": nl}
    flags["cross_mask"] = bool(kpm.any())
    zero = lambda a: not np.any(np.asarray(a))
    if not (zero(inputs["self_in_b"][:nl]) and zero(inputs["cross_in_b"][:nl])):
        raise NotImplementedError("nonzero qkv projection biases not wired up")
    flags["b_o"] = not zero(inputs["self_out_b"][:nl])
    flags["cb_o"] = not zero(inputs["cross_out_b"][:nl])
    flags["b_1"] = not zero(inputs["ffn_b1"][:nl])
    flags["b_2"] = not zero(inputs["ffn_b2"][:nl])
    flags["b_gen"] = not zero(inputs["gen_b"])
    triv = lambda g, b: bool(np.all(np.asarray(g) == 1.0) and np.all(np.asarray(b) == 0.0))
    flags["ln1"] = not triv(inputs["ln1_g"][:nl], inputs["ln1_b"][:nl])
    flags["ln2"] = not triv(inputs["ln2_g"][:nl], inputs["ln2_b"][:nl])
    flags["ln3"] = not triv(inputs["ln3_g"][:nl], inputs["ln3_b"][:nl])
    flags["lnf"] = not triv(inputs["normf_g"], inputs["normf_b"])

    # embeddings (scale folded in on host)
    x0_full = (tok_emb[seq] * f32(np.sqrt(E))).astype(f32)  # [B, LT, E]

    # distance-bias exp table, transposed to [b, h, k, q]; masked entries -> 0
    g = dist_emb[dsq]                                       # [B, q, k, H] f32
    ebv = np.exp(g, dtype=f32)
    causal = np.tril(np.ones((LT, LT), bool))               # [q, k]
    pad = (seq == 0)                                        # [B, LT] (keys)
    masked = (~causal)[None, :, :] | pad[:, None, :]        # [B, q, k]
    ebv *= (~masked)[:, :, :, None]
    ebT = np.ascontiguousarray(ebv.transpose(0, 3, 2, 1)).astype(f16)  # [B,H,k,q]

    siw = np.asarray(inputs["self_in_w"], dtype=f32)
    ciw = np.asarray(inputs["cross_in_w"], dtype=f32)
    sow = np.asarray(inputs["self_out_w"], dtype=f32)
    cow = np.asarray(inputs["cross_out_w"], dtype=f32)
    f1 = np.asarray(inputs["ffn_w1"], dtype=f32)
    f2 = np.asarray(inputs["ffn_w2"], dtype=f32)
    scale = f32(1.0 / np.sqrt(D))

    def packqk(w):   # [nl, 3E, E] -> [nl, E, 2E] = [Wq.T*scale | Wk.T]
        out = np.empty((nl, E, 2 * E), dtype=f16)
        for l in range(nl):
            out[l, :, :E] = (w[l, :E].T * scale).astype(f16)
            out[l, :, E:] = w[l, E:2 * E].T.astype(f16)
        return out

    shared = {
        "wqk": packqk(siw[:nl]),
        "wv": np.ascontiguousarray(siw[:nl, 2 * E:].transpose(0, 2, 1) / 1024.0).astype(f16),
        "wo": np.ascontiguousarray(sow[:nl].transpose(0, 2, 1) * 1024.0).astype(f16),
        "cqk": packqk(ciw[:nl]),
        "cv": np.ascontiguousarray(ciw[:nl, 2 * E:].transpose(0, 2, 1) / 1024.0).astype(f16),
        "co": np.ascontiguousarray(cow[:nl].transpose(0, 2, 1) * 1024.0).astype(f16),
        "w1": np.ascontiguousarray(f1[:nl].transpose(0, 2, 1)).astype(f16),
        "w2": np.ascontiguousarray(f2[:nl].transpose(0, 2, 1)).astype(f16),
        "wgen": np.ascontiguousarray(np.asarray(inputs["gen_w"], dtype=f32).T).astype(f16),
    }
    if flags["b_o"]:
        shared["b_o"] = np.asarray(inputs["self_out_b"][:nl], dtype=f32)
    if flags["cb_o"]:
        shared["cb_o"] = np.asarray(inputs["cross_out_b"][:nl], dtype=f32)
    if flags["b_1"]:
        shared["b_1"] = np.asarray(inputs["ffn_b1"][:nl], dtype=f32)
    if flags["b_2"]:
        shared["b_2"] = np.asarray(inputs["ffn_b2"][:nl], dtype=f32)
    if flags["b_gen"]:
        shared["b_gen"] = np.asarray(inputs["gen_b"], dtype=f32)
    for nm in ("ln1", "ln2", "ln3"):
        if flags[nm]:
            shared[nm + "_g"] = np.asarray(inputs[nm + "_g"][:nl], dtype=f32)
            shared[nm + "_b"] = np.asarray(inputs[nm + "_b"][:nl], dtype=f32)
    if flags["lnf"]:
        shared["lnf_g"] = np.asarray(inputs["normf_g"], dtype=f32)
        shared["lnf_b"] = np.asarray(inputs["normf_b"], dtype=f32)

    # layer-0 self-attention softmax numerators, computed exactly on host:
    # the embedding input is unnormalized (x32), so device fp16 scores would
    # lose ~0.5 absolute score precision and exp would overflow. The scores
    # depend only on the (host-known) embeddings, so exp(s + bias - rowmax)
    # is computed here and streamed as the layer-0 expT table.
    wq0 = siw[0, :E] * scale
    wk0 = siw[0, E:2 * E]
    eb0 = np.empty((B, H, LT, LT), dtype=f16)            # [b, h, k, q]
    for b in range(B):
        qb = (x0_full[b] @ wq0.T).reshape(LT, H, D)
        kb = (x0_full[b] @ wk0.T).reshape(LT, H, D)
        sb = np.einsum("qhd,khd->hqk", qb, kb, optimize=True)  # [H, q, k]
        sb = sb + np.where(masked[b][None], -np.inf, g[b].transpose(2, 0, 1))
        c = sb.max(-1, keepdims=True)
        c = np.where(np.isfinite(c), c, 0.0)
        eb0[b] = np.exp(sb - c).transpose(0, 2, 1)       # -> [h, k, q]
    eb0 = np.nan_to_num(eb0, nan=0.0, posinf=0.0)
    percore = []
    for i in range(NCORES):
        bsl = slice(i * BL, (i + 1) * BL)
        x0c = np.ascontiguousarray(x0_full[bsl].reshape(T, E), dtype=f32)
        m = {
            "x0": x0c,
            "x0T": np.ascontiguousarray(x0c.T).astype(f16),
            "memT": np.ascontiguousarray(memory[bsl].reshape(T, E).T).astype(f16),
            "eb": np.ascontiguousarray(ebT[bsl]),
            "eb0": np.ascontiguousarray(eb0[bsl]),
        }
        if flags["cross_mask"]:
            m["ebc"] = np.ascontiguousarray((~kpm[bsl]).astype(f32))
        percore.append(m)
    return flags, shared, percore


_DBG_RESULTS = None


def kernel(**inputs):
    global _DBG_RESULTS
    flags, shared, percore = _prep_host(inputs)
    flags["dbg_stage"] = int(os.environ.get("CC_DBG", "99"))

    key = tuple(sorted(flags.items()))
    if key not in _CACHE:
        _CACHE[key] = _build_program(flags)
    nc = _CACHE[key]

    in_maps = []
    for i in range(NCORES):
        m = dict(percore[i])
        m.update(shared)
        in_maps.append(m)

    res = run_bass_kernel_spmd(nc, in_maps, core_ids=list(range(NCORES)))
    if flags["dbg_stage"] < 99:
        _DBG_RESULTS = res.results
    out = np.empty((B, LT, V), dtype=np.float32)
    for i in range(NCORES):
        out[i * BL:(i + 1) * BL] = res.results[i]["out"].reshape(BL, LT, V)
    return out

